# revision 9
# baseline (speedup 1.0000x reference)
"""Trainium2 Bass kernel for nn_Encoder_Layer_F (unfold -> grouped 4x4/s2 conv
-> BatchNorm(train) -> LeakyReLU(0.2) -> fold).

Sharding: the 64 locally-connected groups (8x8 patch grid) are split by patch
ROW across the 8 cores (core i owns patch row hp=i). Groups are fully
independent and BN channels belong to exactly one group, so there are no
collectives at all: each core computes its 8 groups x 256 channels over the
full batch, including exact batch statistics.

Per-core program (SPMD, identical on all cores):
  x  [128c, 8wp, 2pr, 2pc, 4qr, 4qc, 32b] parity-quadrant input slab (fp16)
  w  [8wp, 128c, 2zh, 4khx, 4kw, 128z]    weights, kh pre-permuted to the
                                          consumption order [1,2,0,3] and
                                          z-half-major on host
  gb [128zp, 3(invg2/epsg/beta), 8wp, 2zh] f32
  o  [8wp, 128zp, 2zh, 512(oh,ow,b)]  fp16 output (host upcasts to f32)

The conv is 16 PSUM-accumulated matmuls per (group, z-half): contraction over
the 128 input channels, one matmul per 4x4 kernel tap, with the tap's
(oh, ow) range restricted so that zero-padding taps are simply skipped.
fp16 operands; the parity-quadrant x layout makes every tap stream contiguous
runs (strided rhs APs halve PE throughput). Warm matmuls run at theory
(N/2.4GHz start-to-start, LDWEIGHTS hidden).

Schedule rationale (v3, from the 71.7us baseline's and v2's traces):
 - exec_time = last-instr-end - first-useful-instr-start, where the first
   useful instr is the framework const-pool memset just before user code;
   the variable-length framework preamble is excluded, but a fixed ~8us
   end-of-kernel sem-zero postamble is included. The optimizable span is
   [const-memset .. last-out-DMA-receipt].
 - DMA reality (measured): per-queue throughput is ~aggregate/n_queues with
   an aggregate of ~250 (2 queues) to ~340-360 GB/s (3 queues); the consume
   rate is 287 GB/s, so all three DMA-capable engines (sync, gpsimd, scalar)
   carry inputs. Issue cost is ~0.65us per dma_start per engine, and the
   scalar/ACT queue ramps late (act-table loads head its ring), so scalar
   gets only chunks needed after ~+12us.
 - The head is delivery-bound: the PE can start real work only once group
   0's first accumulation set has landed. Splitting the weight layout by
   z-half halves that first-need to 1MB (w[g0,zh0] 512K + x[g0] 512K),
   pulling the first real matmul ~2us earlier. N_WARMUP PE-warmup matmuls
   (~101ns each cold) on a zero tile keep the PE continuously busy from
   ~+1.1us so the HAM clock-gate is at 8/8 when real work starts -- v2
   showed any >1us idle gap in the ramp resets the HAM window and the whole
   mid-kernel runs at 1.2GHz.
 - All out-DMAs ride the sync ring (idle after its input issues); v1 had
   them on scalar where each ~630ns trigger between PRELUs jammed the BN
   pipeline. The last group's zh1 out goes on scalar right after its PRELU.
 - BN chain per (group, zh): bn_stats/bn_aggr (vector) -> Sqrt activation
   with gamma folded in (scalar; host ships 1/gamma^2, eps/gamma^2; Rsqrt
   is blocked by bass and Dsqrt has no act table) -> reciprocal, mean*inv,
   beta-m*inv (vector) -> PRELU drain (scalar).
"""

import numpy as np

import concourse.bass as bass
import concourse.tile as tile
from concourse import bacc, mybir
from concourse.bass_utils import run_bass_kernel_spmd

B = 32
NC = 128
NZ = 256
HP = WP = 8
OK = 4
BN_EPS = 1e-5
LRELU = 0.2

N_WARMUP = 44
OUT_FP16 = True        # device writes fp16 output, host upcasts


# Per-tap valid output range (stride 2, pad 1, kernel 4 on an 8-wide axis):
# i_in = 2*o + k - 1 must lie in [0, 8). k=0 -> o in [1,3]; k=3 -> o in [0,2].
def _tap_range(k):
    lo = 1 if k == 0 else 0
    hi = 2 if k == 3 else 3
    return lo, hi - lo + 1


# Weights are consumed kh-chunk by kh-chunk in this order; the host permutes
# the kh axis so chunk khx on the device is kh = KH_ORDER[khx], making the
# need-ordered weight DMAs contiguous.
KH_ORDER = [1, 2, 0, 3]


def _tap_order():
    # First tap must cover the full (oh, ow) range so that start=True
    # initializes every element of the PSUM accumulation tile.
    taps = []
    for khx, kh in enumerate(KH_ORDER):
        for kw in [1, 0, 2, 3] if kh == 1 else range(4):
            taps.append((khx, kh, kw))
    assert taps[0][1:] == (1, 1)
    return taps


def build_nc():
    f32 = mybir.dt.float32
    mm_dt = mybir.dt.float16
    out_dt = mybir.dt.float16 if OUT_FP16 else f32

    nc = bacc.Bacc(None, target_bir_lowering=False)

    x = nc.declare_dram_parameter("x", [NC, WP, 2, 2, OK, OK, B], mm_dt, isOutput=False)
    w = nc.declare_dram_parameter("w", [WP, NC, 2, 4, 4, 128], mm_dt, isOutput=False)
    gb = nc.declare_dram_parameter("gb", [128, 3, WP, 2], f32, isOutput=False)
    # z-half innermost: both halves of a group leave in one 256KB DMA
    o = nc.declare_dram_parameter("o", [WP, 128, 2, B * OK * OK], out_dt, isOutput=True)

    taps = _tap_order()
    with tile.TileContext(nc) as tc:
        with (
            tc.tile_pool(name="xpool", bufs=8) as xpool,
            tc.tile_pool(name="wpool", bufs=8) as wpool,
            tc.tile_pool(name="psum", bufs=8, space=bass.MemorySpace.PSUM) as psum,
            tc.tile_pool(name="opool", bufs=8) as opool,
            tc.tile_pool(name="spool", bufs=8) as spool,
            tc.tile_pool(name="cpool", bufs=1) as cpool,
        ):
            # --- PE warmup (see module docstring).
            wu = cpool.tile([128, 128], mm_dt)
            nc.vector.memset(wu[:], 0.0)
            ptw = psum.tile([128, OK, OK, B], f32, tag="pt")
            ptwf = ptw.rearrange("p i j b -> p (i j b)")
            for _ in range(N_WARMUP):
                nc.tensor.matmul(ptwf[:, 0:128], wu[:], wu[:],
                                 start=True, stop=True)

            xts, wts = [], []
            for wp in range(WP):
                xts.append(xpool.tile([NC, 2, 2, OK, OK, B], mm_dt,
                                      name=f"xt{wp}", tag="xt"))
                wts.append(wpool.tile([NC, 2, 4, 4, 128], mm_dt,
                                      name=f"wt{wp}", tag="wt"))
            gbt = cpool.tile([128, 3, WP, 2], f32)

            S, G, C = nc.sync, nc.gpsimd, nc.scalar
            # (engine, dst, src) in global need order; per-engine sublists
            # stay need-ordered (HW drains each ring FIFO in order). Scalar
            # only gets chunks needed late (its ACT queue ramps slowly).
            issue = [
                (S, wts[0][:, 0, 0:2], w[0][:, 0, 0:2]),   # g0 zh0 kh=1,2  256K
                (G, xts[0][:, 0:1], x[:, 0, 0:1]),         # g0 pr=0        256K
                (S, xts[0][:, 1:2], x[:, 0, 1:2]),         # g0 pr=1        256K
                (G, wts[0][:, 0, 2:4], w[0][:, 0, 2:4]),   # g0 zh0 kh=0,3  256K
                (S, wts[0][:, 1:2], w[0][:, 1:2]),         # g0 zh1         512K
                (G, wts[1][:, 0:1], w[1][:, 0:1]),         # g1 zh0         512K
                (S, xts[1][:], x[:, 1]),                   # g1 x           512K
                (G, wts[1][:, 1:2], w[1][:, 1:2]),         # g1 zh1         512K
                (C, gbt[:], gb[:]),
                (C, xts[2][:], x[:, 2]),
                (S, wts[2][:, 0:1], w[2][:, 0:1]),
                (G, wts[2][:, 1:2], w[2][:, 1:2]),
                (C, xts[3][:], x[:, 3]),
                (S, wts[3][:], w[3]),                      # 1MB whole group
                (G, wts[4][:], w[4]),
                (C, xts[4][:], x[:, 4]),
                (S, xts[5][:], x[:, 5]),
                (G, wts[5][:], w[5]),
                (C, xts[6][:], x[:, 6]),
                (S, wts[6][:], w[6]),
                (G, wts[7][:], w[7]),
                (C, xts[7][:], x[:, 7]),
            ]
            for eng, dst, src in issue:
                eng.dma_start(dst, src)

            for wp in range(WP):
                xt, wt = xts[wp], wts[wp]
                ot = opool.tile([128, 2, B * OK * OK],
                                mybir.dt.float16 if OUT_FP16 else f32)
                for zh in range(2):
                    # PSUM/output layout is (oh, ow, b) with b innermost.
                    pt = psum.tile([128, OK, OK, B], f32, tag="pt")
                    ptf = pt.rearrange("p i j b -> p (i j b)")
                    for idx, (khx, kh, kw) in enumerate(taps):
                        ol, oc = _tap_range(kh)
                        wl, wc = _tap_range(kw)
                        pr, qr0 = (kh + 1) % 2, ol + (-1 if kh == 0 else (1 if kh == 3 else 0))
                        pc, qc0 = (kw + 1) % 2, wl + (-1 if kw == 0 else (1 if kw == 3 else 0))
                        nc.tensor.matmul(
                            pt[:, ol:ol + oc, wl:wl + wc, :],
                            wt[:, zh, khx, kw, :],
                            xt[:, pr, pc, qr0:qr0 + oc, qc0:qc0 + wc, :],
                            start=(idx == 0),
                            stop=(idx == len(taps) - 1),
                        )

                    st = spool.tile([128, 6], f32)
                    nc.vector.bn_stats(st[:], ptf)
                    mv = spool.tile([128, 2], f32)
                    nc.vector.bn_aggr(mv[:], st[:])
                    # sd = sqrt(var/g^2 + eps/g^2) = sqrt(var+eps)/gamma, so
                    # inv = 1/sd = gamma/sqrt(var+eps) directly after recip.
                    sd = spool.tile([128, 1], f32)
                    nc.scalar.activation(
                        sd[:], mv[:, 1:2], mybir.ActivationFunctionType.Sqrt,
                        bias=gbt[:, 1:2, wp, zh], scale=gbt[:, 0:1, wp, zh],
                    )
                    inv = spool.tile([128, 1], f32)
                    nc.vector.reciprocal(inv[:], sd[:])
                    tmp = spool.tile([128, 1], f32)
                    nc.vector.tensor_mul(tmp[:], mv[:, 0:1], inv[:])
                    sh = spool.tile([128, 1], f32)
                    nc.vector.tensor_sub(sh[:], gbt[:, 2:3, wp, zh], tmp[:])

                    # Prelu(v, alpha) == LeakyReLU(alpha) on TRN2; the Lrelu
                    # func ignores alpha (hardwired 0.01 slope).
                    nc.scalar.activation(
                        ot[:, zh], ptf, mybir.ActivationFunctionType.Prelu,
                        bias=sh[:], scale=inv[:], alpha=LRELU,
                    )
                    if wp == WP - 1:
                        # tail: zh0 ships (sync) while zh1's matmuls run; zh1
                        # ships on scalar right behind its own PRELU.
                        (nc.sync if zh == 0 else nc.scalar).dma_start(
                            o[wp, :, zh:zh + 1], ot[:, zh:zh + 1])
                if wp < WP - 1:
                    # one out-DMA per group on the by-now-idle sync ring
                    nc.sync.dma_start(o[wp], ot[:])

    nc.compile()
    return nc


def shard_inputs(input, weight, gamma, beta):
    """Build the 8 per-core input maps (host-side layout transforms only)."""
    input = np.asarray(input, dtype=np.float32)
    weight = np.asarray(weight, dtype=np.float32)
    gamma = np.asarray(gamma, dtype=np.float32)
    beta = np.asarray(beta, dtype=np.float32)
    io_np = np.float16

    # [B, NC, HP, 4qr, 2pr, WP, 4qc, 2pc] -> [HP, NC, WP, pr, pc, qr, qc, B]
    xs = input.reshape(B, NC, HP, OK, 2, WP, OK, 2).transpose(2, 1, 5, 4, 7, 3, 6, 0)
    xs = np.ascontiguousarray(xs, dtype=io_np)
    # [HP, WP, NZ, NC, 4, 4] -> [HP, WP, NC, zh, kh, kw, 128], kh permuted to
    # the device consumption order KH_ORDER, z-half-major so a group's first
    # accumulation only needs half the group's weights.
    ws = weight.reshape(HP, WP, 2, 128, NC, 4, 4).transpose(0, 1, 4, 2, 5, 6, 3)
    ws = ws[:, :, :, :, KH_ORDER]
    ws = np.ascontiguousarray(ws, dtype=io_np)
    # per (channel): [invg2, epsg, beta] with invg2 = 1/gamma^2, epsg =
    # eps/gamma^2 (gamma folded into the on-device Sqrt activation).
    gs = gamma.reshape(HP, WP, 2, 128).astype(np.float64)
    bs = beta.reshape(HP, WP, 2, 128)
    with np.errstate(divide="ignore"):
        invg2 = (1.0 / (gs * gs)).astype(np.float32)
    epsg = (BN_EPS * invg2.astype(np.float64)).astype(np.float32)
    # [HP, 3, WP, 2, 128] -> [HP, 128, 3, WP, 2]
    gbs = np.stack([invg2, epsg, bs], axis=1).transpose(0, 4, 1, 2, 3)
    gbs = np.ascontiguousarray(gbs, dtype=np.float32)

    return [
        {"x": xs[i], "w": ws[i], "gb": gbs[i]}
        for i in range(HP)
    ]


def unshard_output(results):
    # per-core o: [WP, 128, 2, (oh ow b)] -> full [B, NZ, 32, 32]
    O = np.stack([results[i]["o"] for i in range(HP)])
    O = O.reshape(HP, WP, 128, 2, OK, OK, B)
    O = O.transpose(6, 3, 2, 0, 4, 1, 5).reshape(B, NZ, HP * OK, WP * OK)
    return np.ascontiguousarray(O, dtype=np.float32)


_NC_CACHE = {}


def kernel(input, weight, gamma, beta):
    key = "final"
    if key not in _NC_CACHE:
        _NC_CACHE[key] = build_nc()
    nc = _NC_CACHE[key]
    in_maps = shard_inputs(input, weight, gamma, beta)
    res = run_bass_kernel_spmd(nc, in_maps, list(range(8))).results
    return unshard_output(res)
